# revision 11
# baseline (speedup 1.0000x reference)
"""Capsule routing pooling kernel for Trainium2 (8 NeuronCores, data parallel).

Math: the reference's softmax is over a singleton axis, so the routing
coefficients are identically 1.0 and the routing iterations never affect the
output.  The computation reduces to, per (b, c, 2x2 spatial tile):
    s   = sum of the four D=16 vectors in the tile
    sq  = sum_d s_d^2
    out = s * sqrt(sq) / (1 + sq)

Sharding: batch dim (16) split across 8 cores -> 2 batches/core.  Per core the
(2*64)=128 (b,c) pairs map onto the 128 SBUF partitions; each partition owns a
full 64x64x16 image.

v6 pipeline (bf16 datapath; rel-err budget is 2e-2, bf16 noise is ~4e-3):
  - all loads are SWDGE (gpsimd) DMAs casting f32 -> bf16 in the DMA
    datapath: HBM read bytes unchanged (the floor, ~92us at the measured
    349 GB/s), SBUF writes halved, and DVE tensor_tensor ops get the bf16
    2x_1P perf mode.  2MB load units keep Q7 descriptor-emission short.
    (DMA accum_op+cast was tried for the row-pair add: it wedges the device
    with NRT_EXEC_UNIT_UNRECOVERABLE - do not combine cast with accum.)
  - pooled sums live in ONE persistent SBUF buffer (32KB/partition) so
    fronts never block on store completion and the load stream never stalls
  - super-groups of 4 row-pairs; the PSUM square tile (8KB/partition)
    double-buffers so ACT squares SG i+1 while DVE reduces SG i
  - tail part A (one SG late): ACT square -> DVE reduce over D ->
    scale = sqrt(sq) * 1/(1+sq) (ACT sqrt + DVE fast-reciprocal) -> ACT
    broadcast-expands the bf16 scale to [P,nsg,D]
  - tail part B (two SGs late): DVE multiply s * scale_expanded at bf16 2x
    (the expanded operand keeps the op out of the 1x broadcast mode, and
    the extra SG of lag means DVE never waits on ACT's expand)
  - stores decoupled and batched to ~1MB on the idle sync HWDGE ring
  - output dram tensor is bf16 (half the store traffic); host upcasts
"""

import numpy as np

import concourse.bass as bass
import concourse.bacc as bacc
import concourse.tile as tile
from concourse import mybir
from concourse.bass_utils import run_bass_kernel_spmd

_B, _C, _H, _W, _D = 16, 64, 64, 64, 16
_NCORES = 8
_F32 = mybir.dt.float32
_BF16 = mybir.dt.bfloat16


def _kernel_body(tc, out_ap, in_ap, H, W, D):
    nc = tc.nc
    P = 128
    nH, nW = H // 2, W // 2

    inv4 = in_ap.rearrange("p (q four) w d -> p q (four w d)", four=4)
    inv2 = in_ap.rearrange("p (rp two) w d -> p rp (two w d)", two=2)
    outv = out_ap.rearrange("p y x d -> p y (x d)")

    # super-group schedule in row-pair units: 4s in the bulk (PSUM
    # double-buffering wants nsg <= 128), tiny SGs at the end so the
    # post-last-load drain chain is short.  Coarse 2-row-pair slabs need
    # even alignment, so tapers keep g0 even until the sg=1 entries.
    if nH >= 16:
        sched = [4] * ((nH - 4) // 4) + [2, 1, 1]
    else:
        sched = [nH]
    assert sum(sched) == nH
    nsg_max = max(sched) * nW

    import contextlib

    with contextlib.ExitStack() as ctx:
        slabs = ctx.enter_context(tc.tile_pool(name="slabs", bufs=8))
        rpool = ctx.enter_context(tc.tile_pool(name="rpool", bufs=3))
        sall_pool = ctx.enter_context(tc.tile_pool(name="sall", bufs=1))
        psum = ctx.enter_context(tc.tile_pool(name="psum", bufs=2, space="PSUM"))
        small = ctx.enter_context(tc.tile_pool(name="small", bufs=3))
        smallb = ctx.enter_context(tc.tile_pool(name="smallb", bufs=3))
        scxp = ctx.enter_context(tc.tile_pool(name="scx", bufs=3))

        # persistent pooled-sum buffer for the whole image (bf16, 32KB/part)
        sall = sall_pool.tile([P, nH, nW, D], _BF16, tag="sall")

        def emit_front(sg, g0):
            """loads + row-pair adds + column-pair add for one super-group of
            `sg` row-pairs starting at output row g0; result lands in
            sall[:, g0:g0+sg]."""
            r = rpool.tile([P, 4, nW, 2, D], _BF16, tag="r")
            for li in range(0, sg, 2):
                if sg - li >= 2:
                    assert (g0 + li) % 2 == 0, "coarse slab needs even alignment"
                    t = (g0 + li) // 2
                    slab = slabs.tile([P, 2, 2, nW, 2, D], _BF16, tag="slab")
                    nc.gpsimd.dma_start(
                        out=slab[:],
                        in_=inv4[:, t, :].rearrange(
                            "p (a two b) -> p a two b", a=2, two=2
                        ),
                    )
                    # row-pair sums for 2 row-pairs (DVE bf16 2x, FD=2048)
                    nc.vector.tensor_add(
                        r[:, li : li + 2, :, :, :],
                        slab[:, :, 0, :, :, :],
                        slab[:, :, 1, :, :, :],
                    )
                else:
                    rp = g0 + li
                    slab = slabs.tile([P, 1, 2, nW, 2, D], _BF16, tag="slab")
                    nc.gpsimd.dma_start(
                        out=slab[:],
                        in_=inv2[:, rp, :].rearrange("p (two b) -> p two b", two=2),
                    )
                    nc.vector.tensor_add(
                        r[:, li : li + 1, :, :, :],
                        slab[:, :, 0, :, :, :],
                        slab[:, :, 1, :, :, :],
                    )
            # column-pair add (DVE bf16 2x)
            nc.vector.tensor_add(
                sall[:, g0 : g0 + sg, :, :],
                r[:, 0:sg, :, 0, :],
                r[:, 0:sg, :, 1, :],
            )

        def emit_tail_a(sg, g0):
            """square + reduce + squash scale + ACT expand of the scale.
            Returns the expanded-scale tile view for part B."""
            nsg = sg * nW
            sv = sall[:, g0 : g0 + sg, :, :].rearrange("p s x d -> p (s x) d")
            s2p = psum.tile([P, nsg_max, D], _F32, tag="s2p")
            nc.scalar.activation(
                s2p[:, 0:nsg, :], sv, mybir.ActivationFunctionType.Square
            )
            ch = small.tile([P, nsg_max, 3], _F32, tag="ch")
            scb = smallb.tile([P, nsg_max, 1], _BF16, tag="scb")
            sq = ch[:, 0:nsg, 0:1]
            a = ch[:, 0:nsg, 1:2]
            rec = ch[:, 0:nsg, 2:3]
            sc = scb[:, 0:nsg, 0:1]
            nc.vector.tensor_reduce(
                sq, s2p[:, 0:nsg, :], axis=mybir.AxisListType.X, op=mybir.AluOpType.add
            )
            # scale = sqrt(sq) / (1 + sq)   (1e-8 dropped: sq >= O(1) for
            # this distribution; relative effect <= 1e-6)
            nc.scalar.activation(a, sq, mybir.ActivationFunctionType.Sqrt)
            nc.scalar.add(rec, sq, 1.0)
            nc.vector.reciprocal_approx_fast(rec, rec)
            nc.vector.tensor_mul(sc, a, rec)
            scx = scxp.tile([P, nsg_max, D], _BF16, tag="scx")
            nc.scalar.copy(scx[:, 0:nsg, :], sc.to_broadcast((P, nsg, D)))
            return scx

        def emit_tail_b(sg, g0, scx):
            """in-place broadcast multiply at bf16 2x (expanded scale)."""
            nsg = sg * nW
            sv = sall[:, g0 : g0 + sg, :, :].rearrange("p s x d -> p (s x) d")
            nc.vector.tensor_mul(sv, sv, scx[:, 0:nsg, :])

        def emit_store(y0, y1):
            nc.sync.dma_start(
                out=outv[:, y0:y1, :],
                in_=sall[:, y0:y1, :, :].rearrange("p s x d -> p (s x d)"),
            )

        n = len(sched)
        g0s = []
        g0 = 0
        for sg in sched:
            g0s.append(g0)
            g0 += sg

        stored_to = 0
        done_rows = 0

        def flush_store(min_rows):
            nonlocal stored_to, done_rows
            if done_rows >= min_rows and done_rows > 0:
                emit_store(stored_to, stored_to + done_rows)
                stored_to += done_rows
                done_rows = 0

        scxs = {}
        for si in range(n):
            emit_front(sched[si], g0s[si])
            ai = si - 1  # tail A one SG late
            if ai >= 0:
                scxs[ai] = emit_tail_a(sched[ai], g0s[ai])
            bi = si - 2  # tail B two SGs late
            if bi >= 0:
                emit_tail_b(sched[bi], g0s[bi], scxs.pop(bi))
                done_rows += sched[bi]
                flush_store(8 if si < n - 3 else 2)
        # drain: remaining tail As and Bs
        scxs[n - 1] = emit_tail_a(sched[n - 1], g0s[n - 1])
        for bi in (n - 2, n - 1):
            emit_tail_b(sched[bi], g0s[bi], scxs.pop(bi))
            done_rows += sched[bi]
            flush_store(2 if bi < n - 1 else 1)
        assert stored_to == nH and not scxs


def build_nc(H=_H, W=_W, D=_D):
    """Build and compile the per-core Bass program."""
    nc = bacc.Bacc("TRN2", target_bir_lowering=False, debug=False)
    inp = nc.dram_tensor("inp", [128, H, W, D], _F32, kind="ExternalInput").ap()
    out = nc.dram_tensor(
        "out", [128, H // 2, W // 2, D], _BF16, kind="ExternalOutput"
    ).ap()
    with tile.TileContext(nc) as tc:
        _kernel_body(tc, out, inp, H, W, D)
    nc.compile()
    return nc


_NC_CACHE = {}


def _get_nc():
    if "nc" not in _NC_CACHE:
        _NC_CACHE["nc"] = build_nc()
    return _NC_CACHE["nc"]


def kernel(inp, kernel_size=2, routing_iteration=3, _trace=False, _tmpdir=None):
    inp = np.asarray(inp, dtype=np.float32)
    assert int(kernel_size) == 2, "kernel compiled for kernel_size=2"
    assert inp.shape == (_B, _C, _H, _W, _D), inp.shape
    # routing_iteration is mathematically irrelevant (softmax over singleton
    # axis -> coefficients identically 1); any value >= 1 gives this output.

    nc = _get_nc()
    bpc = _B // _NCORES  # batches per core
    in_maps = [
        {"inp": np.ascontiguousarray(inp[i * bpc : (i + 1) * bpc]).reshape(128, _H, _W, _D)}
        for i in range(_NCORES)
    ]
    res = run_bass_kernel_spmd(
        nc, in_maps, core_ids=list(range(_NCORES)), trace=_trace, tmpdir=_tmpdir
    )
    out = np.empty((_B, _C, _H // 2, _W // 2, _D), dtype=np.float32)
    for i in range(_NCORES):
        out[i * bpc : (i + 1) * bpc] = (
            np.asarray(res.results[i]["out"])
            .astype(np.float32)
            .reshape(bpc, _C, _H // 2, _W // 2, _D)
        )
    if _trace:
        return out, res
    return out
